# revision 20
# baseline (speedup 1.0000x reference)
"""GAT self-attention kernel for Trainium2 (8 NeuronCores, SPMD data-parallel over N).

Math (per graph n):
  h_t = X @ W_t ; q_gate_t = sigmoid(relu(q @ W1_t) @ W2_t)
  s_src_t = (h_t * g1) @ a1 ; s_dst_t = (h_t * g2) @ a2
  score[i,j] = lrelu(s_src_{adj[i,j]-1}[i] + s_dst_{adj[i,j]-1}[j])   (adj>0)
  out = softmax_j(score) @ (h_3 * node_mask)

Device strategy:
  - s_src/s_dst are never formed via full h_t: fold (gate*a) through W_t^T into
    per-graph vectors C[d], then one matmul with X^T gives all s values.
  - The 4-way type select over adj in {0..4} is evaluated as a degree-4
    polynomial in adj with per-partition coefficients (Lagrange through the 4
    type values, p(0)=0). Coefficients are produced directly by folding the
    inverse Vandermonde into the C vectors (M32 matmul), so the per-element
    work is 4 fused DVE ops per [128,512] tile per side.
  - src-side poly is computed in [i-part, j-free] layout, dst-side in
    [j-part, i-free]; PE transposes the src tiles into the dst-layout PSUM
    bank and an identity matmul accumulates the dst poly on top.
  - lrelu(z) = 0.2*z + relu(0.8*z) (ACT relu + one fused DVE op), exp on ACT.
  - masked-exp e = (adj>0)*exp(lrelu(z)) (one fused DVE op); softmax
    normalization via a ones-column appended to h*mask: the final matmul
    emits rowsum as column 300, and the PSUM->SBUF evacuation multiplies by
    its reciprocal.
"""

import numpy as np
from contextlib import ExitStack

import concourse.bass as bass
import concourse.bacc as bacc
import concourse.tile as tile
from concourse import mybir
from concourse import dve_ops
from concourse.dve_spec import Spec, Src0, Src1, C0, C1
from concourse.dve_uop import DveOpSpec
from concourse.bass_utils import run_bass_kernel_spmd


def _register_dve_op(name, spec):
    """Runtime-register a custom DVE op (fp32-internal fused pipeline)."""
    if name in dve_ops._SUB_OPCODE_FOR_NAME:
        return dve_ops.CUSTOM_DVE_SPECS[name + "_OP"]
    op = dve_ops.DveOp(name, spec, subdim=False, uops_sha={})
    dve_ops.OPS.append(op)
    dve_ops.CUSTOM_DVE_SPECS[name] = spec
    dve_ops._SUB_OPCODE_FOR_NAME[name] = (
        max(dve_ops._SUB_OPCODE_FOR_NAME.values()) + 1)
    shas = {}
    for ver in ("v3", "v4"):
        s = DveOpSpec(
            name=name,
            opcode=dve_ops.get_dve_sub_opcode(name),
            uops=dve_ops.lower(spec, ver=ver),
            rd1_en=dve_ops.has_src1(spec),
        ).sha(ver)
        shas[ver] = s
    object.__setattr__(op, "uops_sha", shas)
    dve_ops.CUSTOM_DVE_SPECS[name + "_OP"] = op
    return op


def _register_horner():
    # out = (in0*in1 + s0)*in1 + s1 : cubic tail given t1 = a3*z + a2
    from concourse.dve_spec import relu as _relu
    return _register_dve_op("HORNER2A_ANT", Spec(
        body=(Src0 * Src1 + C0) * Src1 + C1,
        reference=lambda in0, in1, s0, s1, imm2: (in0 * in1 + s0) * in1 + s1,
    ))


def _register_lrelu():
    # out = s0*in0 + relu(s1*in1); with in0 == in1 == z and s0=0.2, s1=0.8
    # this is leaky-relu(z) in a single pass
    from concourse.dve_spec import relu as _relu
    return _register_dve_op("LRELU_ANT", Spec(
        body=Src0 * C0 + _relu(Src1 * C1),
        reference=lambda in0, in1, s0, s1, imm2: in0 * s0 + np.maximum(in1 * s1, 0),
    ))

f32 = mybir.dt.float32
f32r = mybir.dt.float32r
bf16 = mybir.dt.bfloat16


def _r(ap):
    # reinterpret an f32 AP as float32r: single-pass PE matmul (vs 4-pass f32)
    return ap.bitcast(f32r)
Alu = mybir.AluOpType
Act = mybir.ActivationFunctionType

N, E, D, NT = 32, 512, 300, 4
D2 = 2 * D  # 600
NCORES = 8
GPC = N // NCORES  # graphs per core
SLOPE = 0.2

DC3 = [(0, 128), (128, 128), (256, 44)]          # 300 split into <=128 chunks
EC4 = [(i * 128, 128) for i in range(4)]          # 512 split into 4 chunks


def _vinv():
    # centered basis z = adj - 2.5: coeffs a0..a3 of the cubic through
    # (z_t, u_t), z_t in {-1.5,-0.5,0.5,1.5} (well conditioned, exact bf16)
    V = np.array([[((t + 1) - 2.5) ** m for m in range(4)] for t in range(4)],
                 np.float64)
    return np.linalg.inv(V)


def build_nc():
    nc = bacc.Bacc("TRN2", target_bir_lowering=False, debug=False)

    def din(name, shape, dt=f32):
        return nc.dram_tensor(name, shape, dt, kind="ExternalInput").ap()

    xT = din("xT", [GPC, 384, E], f32r)            # input_state[n].T
    adjA = din("adjA", [GPC, E, E], bf16)  # adj as bf16
    adjB = din("adjB", [GPC, E, E], bf16)  # adj.T as bf16
    nmask = din("nmask", [GPC, 4, 128])
    qT = din("qT", [D, GPC], bf16)               # query_vec.T
    w3 = din("w3", [384, D], f32r)                 # W_type[3] as stored [din, f]
    wT = din("wT", [NT, D, D], f32r)             # W_type[t].T  [f, din]
    w1 = din("w1", [NT, D, D2], bf16)
    w2 = din("w2", [NT, D2, D2], bf16)
    ave = din("ave", [3, 128, 32])        # a_type * Vinv[k,t], per side
    ident = din("ident", [128, 128])
    out = nc.dram_tensor("out", [GPC, E, D], f32, kind="ExternalOutput").ap()

    with tile.TileContext(nc) as tc:
        with ExitStack() as ctx:
            _body(ctx, tc, xT, adjA, adjB, nmask, qT, w3, wT, w1, w2, ave,
                  ident, out)
    nc.compile()
    return nc


def _body(ctx, tc, xT, adjA, adjB, nmask, qT, w3, wT, w1, w2, ave, ident, out):
    nc = tc.nc
    HORNER = _register_horner()
    LRELU = _register_lrelu()
    const = ctx.enter_context(tc.tile_pool(name="const", bufs=1))
    wstream = ctx.enter_context(tc.tile_pool(name="wstream", bufs=2))
    prep = ctx.enter_context(tc.tile_pool(name="prep", bufs=2))
    prep_keep = ctx.enter_context(tc.tile_pool(name="prep_keep", bufs=1))
    xpool = ctx.enter_context(tc.tile_pool(name="xpool", bufs=4))
    adjp = ctx.enter_context(tc.tile_pool(name="adjp", bufs=2))
    adjtp = ctx.enter_context(tc.tile_pool(name="adjtp", bufs=2))
    polyp = ctx.enter_context(tc.tile_pool(name="polyp", bufs=2))
    srcp = ctx.enter_context(tc.tile_pool(name="srcp", bufs=5))
    ehp = ctx.enter_context(tc.tile_pool(name="ehp", bufs=6))
    hmp = ctx.enter_context(tc.tile_pool(name="hmp", bufs=17))
    ckp = ctx.enter_context(tc.tile_pool(name="ckp", bufs=6))
    outp = ctx.enter_context(tc.tile_pool(name="outp", bufs=3))
    otp = ctx.enter_context(tc.tile_pool(name="otp", bufs=2))
    ps = ctx.enter_context(tc.tile_pool(name="ps", bufs=2, space="PSUM"))

    # ---- per-core constants ----
    ID = const.tile([128, 128], f32)
    nc.sync.dma_start(out=ID, in_=ident)
    IDR = const.tile([128, 128], f32r)
    nc.sync.dma_start(out=IDR, in_=ident.bitcast(f32r))
    IDB = const.tile([128, 128], bf16)
    nc.scalar.activation(IDB, ID, Act.Copy)
    W3c = const.tile([128, 3, D], f32r)          # d padded to 384
    nc.sync.dma_start(out=W3c, in_=w3.rearrange("(c p) f -> p c f", p=128))
    QTc = []
    for ci, (off, ln) in enumerate(DC3):
        t_ = const.tile([128, GPC], bf16, tag=f"qt_{off}")
        nc.sync.dma_start(out=t_[:ln, :], in_=qT[off:off + ln, :])
        QTc.append(t_)
    AVE = []
    for ci in range(3):
        a_ = prep_keep.tile([128, 32], f32, tag=f"ave_{ci}")
        nc.sync.dma_start(out=a_, in_=ave[ci])
        AVE.append(a_)

    # ---- phase 1: X^T loads + h matmuls + hm for every graph (PE warm) ----
    XTs = []
    HMs = []
    for n in range(GPC):
        XT = xpool.tile([128, 3, E], f32r, tag=f"xt_{n}")
        nc.sync.dma_start(out=XT, in_=xT[n].rearrange("(c p) e -> p c e", p=128))
        XTs.append(XT)
        NM4 = outp.tile([128, 4], f32, tag="nm")
        nc.sync.dma_start(out=NM4, in_=nmask[n].rearrange("c p -> p c"))
        HM = []
        for ii, (eo, el) in enumerate(EC4):
            psh = ps.tile([128, D], f32, tag="pa")
            for ci in range(3):
                nc.tensor.matmul(psh, XT[:, ci, eo:eo + el], W3c[:, ci, :],
                                 start=(ci == 0), stop=(ci == 2))
            hm = hmp.tile([128, 304], bf16, tag="hm")
            nc.vector.memset(hm[:, D:D + 2], 1.0)
            nc.scalar.mul(hm[:, 0:D], psh, NM4[:, ii:ii + 1])
            HM.append(hm)
        HMs.append(HM)

    # ---- q-gate path (per type) ----
    QGT = {}
    for t in range(NT):
        W1c = []
        for ci, (off, ln) in enumerate(DC3):
            t_ = wstream.tile([128, D2], bf16, tag=f"w1c_{off}")
            nc.sync.dma_start(out=t_[:ln, :], in_=w1[t, off:off + ln, :])
            W1c.append(t_)
        ps1a = ps.tile([GPC, D], f32, tag="pa")
        ps1b = ps.tile([GPC, D], f32, tag="pb")
        for ci, (off, ln) in enumerate(DC3):
            nc.tensor.matmul(ps1a, QTc[ci][:ln, :], W1c[ci][:ln, 0:D],
                             start=(ci == 0), stop=(ci == 2))
        for ci, (off, ln) in enumerate(DC3):
            nc.tensor.matmul(ps1b, QTc[ci][:ln, :], W1c[ci][:ln, D:D2],
                             start=(ci == 0), stop=(ci == 2))
        R1 = prep.tile([GPC, D2], bf16, tag="r1")
        nc.scalar.activation(R1[:, 0:D], ps1a, Act.Relu)
        nc.scalar.activation(R1[:, D:D2], ps1b, Act.Relu)
        # transpose R1 -> R1T chunks [<=128, GPC]
        OC6 = [(s, off, ln) for s in range(2) for (off, ln) in DC3]
        R1T = []
        for s, off, ln in OC6:
            pst = ps.tile([128, GPC], bf16, tag="po")
            nc.tensor.matmul(pst[:ln, :], R1[:, s * D + off:s * D + off + ln],
                             IDB[:GPC, :GPC], is_transpose=True,
                             start=True, stop=True)
            sb = prep.tile([128, GPC], bf16, tag=f"r1t_{s}_{off}")
            nc.vector.tensor_copy(sb[:ln, :], pst[:ln, :])
            R1T.append((sb, off, ln, s))
        # layer 2: stationary r1T chunks, moving W2 rows -> qg [GPC, 600]
        ps2a = ps.tile([GPC, D], f32, tag="pa")
        ps2b = ps.tile([GPC, D], f32, tag="pb")
        for ki, (sb, off, ln, s) in enumerate(R1T):
            w2c = wstream.tile([128, D2], bf16, tag=f"w2c_{s}_{off}")
            nc.sync.dma_start(out=w2c[:ln, :],
                              in_=w2[t, s * D + off:s * D + off + ln, :])
            nc.tensor.matmul(ps2a, sb[:ln, :], w2c[:ln, 0:D],
                             start=(ki == 0), stop=(ki == 5))
            nc.tensor.matmul(ps2b, sb[:ln, :], w2c[:ln, D:D2],
                             start=(ki == 0), stop=(ki == 5))
        SG = prep.tile([GPC, D2], bf16, tag="sg")
        nc.scalar.activation(SG[:, 0:D], ps2a, Act.Sigmoid)
        nc.scalar.activation(SG[:, D:D2], ps2b, Act.Sigmoid)
        # transpose qg -> [<=128, GPC] chunks
        for s, off, ln in OC6:
            pst = ps.tile([128, GPC], bf16, tag="po")
            nc.tensor.matmul(pst[:ln, :], SG[:, s * D + off:s * D + off + ln],
                             IDB[:GPC, :GPC], is_transpose=True,
                             start=True, stop=True)
            qg = prep_keep.tile([128, GPC], bf16, tag=f"qg_{t}_{s}_{off}")
            nc.vector.tensor_copy(qg[:ln, :], pst[:ln, :])
            QGT[(t, s, off)] = (qg, ln)

    # ---- gza' tiles and C' fold (see module docstring) ----
    psCp = ps.tile([32, D], f32, tag="po")
    mm_i = 0
    for t in range(NT):
        GZAp = []
        for ci, (off, ln) in enumerate(DC3):
            g_ = prep.tile([128, 32], f32r, tag=f"gzap_{off}")
            GZAp.append(g_)
        for s in range(2):
            for ci, (off, ln) in enumerate(DC3):
                qg, _ = QGT[(t, s, off)]
                qg_ap = qg[:ln, :]
                qg_rep = bass.AP(tensor=qg_ap.tensor, offset=qg_ap.offset,
                                 ap=[qg_ap.ap[0], [0, 4], qg_ap.ap[1]])
                a_ap = AVE[ci][:ln, t * 8 + s * 4:t * 8 + s * 4 + 4]
                a_rep = bass.AP(tensor=a_ap.tensor, offset=a_ap.offset,
                                ap=[a_ap.ap[0], a_ap.ap[1], [0, 4]])
                o_ap = GZAp[ci][:ln, s * 4:s * 4 + 28]
                o_rep = bass.AP(tensor=o_ap.tensor, offset=o_ap.offset,
                                ap=[o_ap.ap[0], [8, 4], [1, 4]])
                nc.vector.tensor_mul(o_rep, qg_rep, a_rep)
        for ci, (off, ln) in enumerate(DC3):
            wt_ = wstream.tile([128, D], f32r, tag="wTc")
            nc.sync.dma_start(out=wt_[:ln, :], in_=wT[t, off:off + ln, :])
            nc.tensor.matmul(psCp, GZAp[ci][:ln, :], wt_[:ln, :],
                             start=(mm_i == 0), stop=(mm_i == NT * 3 - 1))
            mm_i += 1
    SBC2 = prep.tile([32, D], f32, tag="sbc2")
    nc.scalar.copy(SBC2, psCp)
    CP = []
    for ci, (off, ln) in enumerate(DC3):
        pst = ps.tile([128, 32], f32, tag="po")
        nc.tensor.matmul(pst[:ln, :], SBC2[:, off:off + ln], ID[:32, :32],
                         is_transpose=True, start=True, stop=True)
        sb = prep_keep.tile([128, 32], f32r, tag=f"cp_{off}")
        nc.vector.memset(sb.bitcast(f32), 0.0)
        nc.vector.tensor_copy(sb[:ln, :], pst[:ln, :])
        CP.append(sb)

    # ---- per graph: coefficients, scores, softmax, output ----
    for n in range(GPC):
        XT = XTs[n]
        HM = HMs[n]
        CK = []
        for ii, (eo, el) in enumerate(EC4):
            psk = ps.tile([128, 8], f32, tag="pb")
            for ci in range(3):
                cp_n = CP[ci][:, n::GPC]   # 8 cols: (k,s) -> 2k+s
                nc.tensor.matmul(psk, XT[:, ci, eo:eo + el], cp_n,
                                 start=(ci == 0), stop=(ci == 2))
            ck = ckp.tile([128, 8], f32, tag="ck")
            nc.vector.tensor_copy(ck, psk)
            CK.append(ck)

        # A-side: p_src per i-tile (z = adj - 2.5, cubic in z)
        AJ4 = adjp.tile([128, 4, E], bf16, tag="aj")
        nc.gpsimd.dma_start(out=AJ4, in_=adjA[n].rearrange("(c p) e -> p c e", p=128))
        PS_I = []
        for ii, (eo, el) in enumerate(EC4):
            aj = AJ4[:, ii, :]
            ck = CK[ii]
            t1 = polyp.tile([128, E], bf16, tag="t1")
            nc.vector.tensor_scalar(t1, aj, ck[:, 6:7], ck[:, 4:5], Alu.mult, Alu.add)
            pi = srcp.tile([128, E], bf16, tag="pi")
            nc.vector._custom_dve(HORNER, out=pi, in0=t1, in1=aj,
                                  s0=ck[:, 2:3], s1=ck[:, 0:1])
            PS_I.append(pi)

        # B-side + combine + exp per j-tile
        EH = []
        AT4 = adjtp.tile([128, 4, E], bf16, tag="at")
        nc.gpsimd.dma_start(out=AT4, in_=adjB[n].rearrange("(c p) e -> p c e", p=128))
        for jj, (eo, el) in enumerate(EC4):
            at = AT4[:, jj, :]
            zt = ps.tile([128, E], bf16, tag="pz")
            for ii in range(4):
                nc.tensor.matmul(zt[:, ii * 128:(ii + 1) * 128],
                                 PS_I[ii][:, eo:eo + el], IDB,
                                 is_transpose=True, start=(ii == 0),
                                 stop=(ii == 3), skip_group_check=True)
            ck = CK[jj]
            t1 = polyp.tile([128, E], bf16, tag="t1b")
            nc.vector.tensor_scalar(t1, at, ck[:, 7:8], ck[:, 5:6], Alu.mult, Alu.add)
            pb = polyp.tile([128, E], bf16, tag="pb")
            nc.vector._custom_dve(HORNER, out=pb, in0=t1, in1=at,
                                  s0=ck[:, 3:4], s1=ck[:, 1:2])
            zs = polyp.tile([128, E], bf16, tag="zs")
            nc.vector.scalar_tensor_tensor(zs, zt, 0.0, pb, Alu.add, Alu.add)
            # lrelu = 0.2*z + relu(0.8*z) in one fused op; exp; mask (z>-2)
            lr = polyp.tile([128, E], bf16, tag="lr")
            nc.vector._custom_dve(LRELU, out=lr, in0=zs, in1=zs,
                                  s0=SLOPE, s1=0.8)
            ex = polyp.tile([128, E], bf16, tag="ex")
            nc.scalar.activation(ex, lr, Act.Exp)
            eh = ehp.tile([128, E], bf16, tag="eh")
            nc.vector.scalar_tensor_tensor(eh, at, -2.0, ex, Alu.is_gt, Alu.mult)
            EH.append(eh)

        # final matmul + normalize
        OT = otp.tile([128, 4, D], f32, tag="ot")
        for ii, (eo, el) in enumerate(EC4):
            po = ps.tile([128, D + 2], f32, tag="po")
            for jj in range(4):
                nc.tensor.matmul(po, EH[jj][:, eo:eo + el], HM[jj][:, 0:D + 2],
                                 start=(jj == 0), stop=(jj == 3))
            rc = outp.tile([128, 1], f32, tag="rc")
            nc.vector.reciprocal(rc, po[:, D:D + 1])
            nc.scalar.mul(OT[:, ii, :], po[:, 0:D], rc)
        nc.sync.dma_start(out=out[n].rearrange("(c p) d -> p c d", p=128), in_=OT)


def _prep_inputs(input_state, adj, node_mask, query_vec, W_type, a_type,
                 qattn_W1, qattn_W2):
    import ml_dtypes
    X = np.asarray(input_state, np.float32)
    A = np.asarray(adj, np.int32)
    NMsk = np.asarray(node_mask, np.float32)
    Q = np.asarray(query_vec, np.float32)
    W = np.ascontiguousarray(np.asarray(W_type, np.float32))
    AV = np.asarray(a_type, np.float32)
    W1 = np.ascontiguousarray(np.asarray(qattn_W1, np.float32))
    W2 = np.ascontiguousarray(np.asarray(qattn_W2, np.float32))
    wT = np.ascontiguousarray(W.transpose(0, 2, 1))
    w3 = np.zeros((384, D), np.float32)
    w3[:D] = W[NT - 1]
    Vi = _vinv()  # [k, t]
    # ave0[t, s, f, k] = a[t, s*300+f] * Vinv[k, t]; relaid to
    # [chunk, 128, (t,s,k)] with zero padding in the 44-row chunk
    ave0 = (AV.reshape(NT, 2, D, 1).astype(np.float64) *
            Vi.T.reshape(NT, 1, 1, 4)).astype(np.float32)
    ave = np.zeros((3, 128, 32), np.float32)
    for ci, (off, ln) in enumerate(DC3):
        for t in range(NT):
            for s in range(2):
                ave[ci, :ln, t * 8 + s * 4:t * 8 + s * 4 + 4] = ave0[t, s, off:off + ln, :]
    W1b = np.ascontiguousarray(W1.astype(ml_dtypes.bfloat16))
    W2b = np.ascontiguousarray(W2.astype(ml_dtypes.bfloat16))
    ident = np.ascontiguousarray(np.eye(128, dtype=np.float32))
    in_maps = []
    for c in range(NCORES):
        sl = slice(c * GPC, (c + 1) * GPC)
        Ac = A[sl]
        in_maps.append({
            "xT": np.concatenate(
                [X[sl].transpose(0, 2, 1),
                 np.zeros((GPC, 384 - D, E), np.float32)], axis=1),
            "adjA": np.ascontiguousarray(
                (Ac.astype(np.float32) - 2.5).astype(ml_dtypes.bfloat16)),
            "adjB": np.ascontiguousarray(
                (Ac.transpose(0, 2, 1).astype(np.float32) - 2.5)
                .astype(ml_dtypes.bfloat16)),
            "nmask": np.ascontiguousarray(
                NMsk[sl, :, 0].reshape(GPC, 4, 128)),
            "qT": np.ascontiguousarray(Q[sl].T.astype(ml_dtypes.bfloat16)),
            "w3": w3,
            "wT": wT,
            "w1": W1b,
            "w2": W2b,
            "ave": ave,
            "ident": ident,
        })
    return in_maps


_NC_CACHE = {}


def kernel(**inputs):
    if "nc" not in _NC_CACHE:
        _NC_CACHE["nc"] = build_nc()
    nc = _NC_CACHE["nc"]
    in_maps = _prep_inputs(**inputs)
    res = run_bass_kernel_spmd(nc, in_maps, list(range(NCORES)))
    outs = [np.asarray(res.results[c]["out"]) for c in range(NCORES)]
    return np.concatenate(outs, axis=0).astype(np.float32)
